# revision 3
# baseline (speedup 1.0000x reference)
"""Trainium2 Bass kernel for a 2-step BasicNCA2D cell update (fp8 DoubleRow).

Strategy
--------
Data-parallel over batch: 8 images, one per NeuronCore. Per core the two NCA
steps are fused on-chip (x never round-trips to DRAM between steps).

Per step the math is
    y  = depthwise_conv5x5(x, conv_w) + conv_b        (reflect padding)
    h  = relu([x, y] @ fc0_w + fc0_b)
    dx = h @ fc1_w
    x' = concat([x[..., :1], x[..., 1:] + dx])

conv+fc0 are fused into accumulating fp8 DoubleRow matmuls over shifted
4-row x blocks (block k = image rows 4k-2..4k+1, 129 blocks/stage):
    h_pre[group g = rows 4g..4g+3] = sum_{dj} DR(WAB[dj], ring[k=g,g+1])
at 0.5 cycles/out-col -> 5 matmuls x 256 cycles per 4-row group.

NEW vs the previous version: the whole residual path lives in the SHIFTED
block layout, and fc1 also runs as one fp8 DoubleRow matmul per block:
  - h = relu(h_pre) is written by ACT directly as fp8 with a per-hidden-unit
    scale sigma_m folded into the relu scale/bias (per-partition APs), into a
    17-slot h ring (+ permanent zero slot 0, + dup slot 17 for pair wraps).
  - fc1 for shifted block k contracts K=256 = the (h(k-1), h(k)) slot pair in
    one DoubleRow matmul whose stationary absorbs the +-2-row misalignment:
    slab A uses only h(k-1) rows 2..3 (out rows 4k-2,4k-1), slab B only
    h(k) rows 0..1, with zero-padding elsewhere. fp8 weights are
    q8(fc1_w[m,:]/sigma_m).  107ns vs 213ns for the old bf16 fc1.
  - dx lands in SHIFTED-block PSUM, so the residual add (DVE) and the next
    stage's fp8 ring production (one aligned Pool tensor_copy per block,
    instead of two partition-shifted halves) are both single ops. x0's bf16
    residual is uploaded pre-shifted (XSH); the output is written shifted
    and un-shifted host-side.
  - fc1 outputs for adjacent blocks (2k, 2k+1) share one 2-bank PSUM tile so
    the residual add handles two blocks per DVE instruction (amortizing the
    PSUM access latency).

Engine budget per pass (cost model): PE 164us (conv 136 + fc1 28) is the
bottleneck; ACT 157us (256 relus), DVE ~154us (129 paired adds), Pool
~120us (65 paired ring copies + halos + dups). I/O: X8D fp8 ring upload,
XSH bf16 shifted residual, YD bf16 shifted output.

Accuracy: fp8 h/fc1 with weight-optimized per-unit scales adds ~1.1e-2 in
quadrature to the 1.30e-2 fp8-conv-path error -> 1.72e-2 (budget 2e-2),
validated in fp64 simulation over the full batch (steps=1: 1.05e-2).
"""

import numpy as np
import ml_dtypes

import concourse.mybir as mybir
import concourse.tile as tile
from concourse import bacc
from concourse.bass_utils import run_bass_kernel_spmd

F32 = mybir.dt.float32
F8 = mybir.dt.float8e4
BF16 = mybir.dt.bfloat16
DR = mybir.MatmulPerfMode.DoubleRow

H = 512
W = 512
C = 24
CP = 32
HD = 32
NCORES = 8
NGRP = H // 4          # 128 aligned conv groups of 4 rows per stage
NBLK = NGRP + 1        # 129 shifted blocks per stage
NBATCH = NBLK // 4     # 32 full 4-block DMA batches (+1 tail block)
RP = 520               # stage-0 ring slot pitch (516 used)
R1P = 516              # stage>=1 ring slot pitch
SCALE = 512.0


def _build_nc(steps: int, repeat: int = 1):
    nc = bacc.Bacc("TRN2", target_bir_lowering=False, debug=False)

    X8D = nc.dram_tensor("X8D", [NBATCH + 1, 4, CP, 4, RP], F8, kind="ExternalInput")
    XSH = nc.dram_tensor("XSH", [NBATCH + 1, 4, CP, 4, W], BF16, kind="ExternalInput")
    WAB8 = nc.dram_tensor("WAB8", [128, 5, 2, 128], F8, kind="ExternalInput")
    WCS = nc.dram_tensor("WCS", [128, 2, 128], F8, kind="ExternalInput")
    WCZ = nc.dram_tensor("WCZ", [128, 2, 128], F8, kind="ExternalInput")
    BIAS = nc.dram_tensor("BIAS", [128, 1], F32, kind="ExternalInput")
    SCL = nc.dram_tensor("SCL", [128, 1], F32, kind="ExternalInput")
    YD = nc.dram_tensor("YD", [NBATCH + 1, 4, CP, 4, W], BF16, kind="ExternalOutput")

    last = steps - 1

    with tile.TileContext(nc) as tc:
        with (
            tc.tile_pool(name="wpool", bufs=1) as wpool,
            tc.tile_pool(name="rpool", bufs=1) as rpool,
            tc.tile_pool(name="xpool", bufs=1) as xpool,
            tc.tile_pool(name="hpool", bufs=1) as hpool,
            tc.tile_pool(name="pp", bufs=4, space="PSUM") as pp,
            tc.tile_pool(name="ppdx", bufs=2, space="PSUM") as ppdx,
        ):
            wab_t = wpool.tile([128, 5, 2, 128], F8, tag="wab")
            nc.sync.dma_start(wab_t[:], WAB8.ap())
            wcs_t = wpool.tile([128, 2, 128], F8, tag="wcs")
            nc.sync.dma_start(wcs_t[:], WCS.ap())
            wcz_t = wpool.tile([128, 2, 128], F8, tag="wcz")
            nc.sync.dma_start(wcz_t[:], WCZ.ap())
            bias_t = wpool.tile([128, 1], F32, tag="bias")
            nc.sync.dma_start(bias_t[:], BIAS.ap())
            scl_t = wpool.tile([128, 1], F32, tag="scl")
            nc.sync.dma_start(scl_t[:], SCL.ap())

            # stage-0 x ring: 17-slot fp8 (slot16 = DMA'd dup of slot 0);
            # stage>=1 rings: linear 129 slots, fp8 of the shifted residual
            rings = [rpool.tile([128, 17, RP], F8, tag="r0", name="r0")]
            for s in range(1, steps):
                rings.append(
                    rpool.tile([128, NBLK, R1P], F8, tag=f"r{s}", name=f"r{s}")
                )
            # h rings: slot 0 = permanent zeros, 1..16 rotate (h(g) at
            # 1 + g%16), slot 17 = dup of slot 1 for pair wraps
            hs = [
                hpool.tile([128, 18, W], F8, tag=f"h{s}", name=f"h{s}")
                for s in range(steps)
            ]
            # per-stage bf16 residual rings in SHIFTED block layout
            xsh = [
                xpool.tile([128, 16, W], BF16, tag=f"xsh{s}", name=f"xsh{s}")
                for s in range(steps)
            ]
            outb = xpool.tile([128, 16, W], BF16, tag="outb")

            for t in hs:
                nc.gpsimd.memset(t[:, 0, :], 0.0)

            def load_batch(j):
                """Stage-0 DMA loads: X8D + XSH blocks 4j..4j+3 (+dups)."""
                if j > NBATCH:
                    return
                if j < NBATCH:
                    s0 = (4 * j) % 16
                    nc.sync.dma_start(rings[0][:, s0 : s0 + 4, :], X8D.ap()[j])
                    nc.sync.dma_start(xsh[0][:, s0 : s0 + 4, :], XSH.ap()[j])
                else:
                    # tail: bf16 block 128 -> slot 0
                    nc.sync.dma_start(xsh[0][:, 0, :], XSH.ap()[NBATCH, :, :, 0, :])
                if j > 0 and j % 4 == 0:
                    # block 4j lands in ring slot 0; duplicate into slot 16
                    nc.sync.dma_start(rings[0][:, 16, :], X8D.ap()[j, :, :, 0, :])

            def part1(s, g):
                """conv+fc0 (5 DoubleRow fp8 matmuls) + fp8 relu, group g."""
                ring = rings[s]
                sg = g % 16 if s == 0 else g
                hp = pp.tile([128, W], F32, tag="hp", name=f"hp{s}_{g}")
                for dj in range(5):
                    nc.tensor.matmul(
                        hp[:],
                        wab_t[:, dj],
                        ring[:, sg : sg + 2, dj : dj + W],
                        start=(dj == 0),
                        stop=(dj == 4),
                        perf_mode=DR,
                    )
                nc.scalar.activation(
                    hs[s][:, 1 + g % 16, :], hp[:],
                    mybir.ActivationFunctionType.Relu,
                    bias=bias_t[:], scale=scl_t[:],
                )

            def halo_cols_batch(r, s0, nslot):
                # reflect halo cols for ring slots s0..s0+nslot-1:
                # cols (0,514)<-(4,512) and (1,515)<-(3,511), strided APs
                src = r[:, s0 : s0 + nslot, :]
                for vc, pc in ((0, 4), (1, 3)):
                    nc.gpsimd.tensor_copy(
                        src[:, :, vc : vc + 515 : 514],
                        src[:, :, pc : pc + 509 : 508],
                    )

            pair_dx = [None] * steps

            def part2(s, k):
                """fc1 (shift-absorbing fp8 DR) + residual add (+ ring/DMA)."""
                hr = hs[s]
                if k >= 16 and k % 16 == 0 and k < NBLK - 1:
                    # pair (h(k-1)@16, h(k)@17): dup h(k) from slot 1 to 17
                    nc.gpsimd.tensor_copy(hr[:, 17, :], hr[:, 1, :])
                ps = 0 if k == 0 else 1 + ((k - 1) % 16)
                wt = wcz_t if k == NBLK - 1 else wcs_t
                sub = k % 2
                if sub == 0:
                    pair_dx[s] = ppdx.tile(
                        [128, 2, W], F32, tag="dx", name=f"dx{s}_{k // 2}"
                    )
                dxp = pair_dx[s]
                nc.tensor.matmul(
                    dxp[:, sub, :], wt, hr[:, ps : ps + 2, :],
                    start=True, stop=True, perf_mode=DR,
                )
                if not (sub == 1 or k == NBLK - 1):
                    return
                # flush the pair (or final singleton) with one paired add
                n = sub + 1
                k0 = k - sub
                s0 = k0 % 16
                dst = outb if s == last else xsh[s + 1]
                nc.vector.tensor_add(
                    dst[:, s0 : s0 + n, :],
                    dxp[:, 0:n, :],
                    xsh[s][:, s0 : s0 + n, :],
                )
                if s < last:
                    r = rings[s + 1]
                    nc.gpsimd.tensor_copy(
                        r[:, k0 : k0 + n, 2 : 2 + W], dst[:, s0 : s0 + n, :]
                    )
                    if k % 4 == 3:
                        halo_cols_batch(r, k - 3, 4)
                        if k == 3:
                            # block 0 reflect rows: row -2 := row 2 (block 1
                            # parts 0:32), row -1 := row 1 (own parts 96:128)
                            nc.gpsimd.tensor_copy(
                                r[0:32, 0, 0:R1P], r[0:32, 1, 0:R1P]
                            )
                            nc.gpsimd.tensor_copy(
                                r[32:64, 0, 0:R1P], r[96:128, 0, 0:R1P]
                            )
                    if k == NBLK - 1:
                        kb = NBLK - 1
                        halo_cols_batch(r, kb, 1)
                        # block 128 reflect rows: 512 := 510, 513 := 509
                        nc.gpsimd.tensor_copy(
                            r[64:96, kb, 0:R1P], r[0:32, kb, 0:R1P]
                        )
                        nc.gpsimd.tensor_copy(
                            r[96:128, kb, 0:R1P], r[96:128, kb - 1, 0:R1P]
                        )
                else:
                    if k >= NBLK - 5:
                        # tail blocks: per-block DMAs to shorten the drain
                        for kk in range(k0, k + 1):
                            nc.sync.dma_start(
                                YD.ap()[kk // 4, :, :, kk % 4, :],
                                outb[:, kk % 16, :],
                            )
                    elif k % 4 == 3:
                        nc.sync.dma_start(
                            YD.ap()[k // 4], outb[:, (k - 3) % 16 : (k - 3) % 16 + 4, :]
                        )

            for _rep in range(repeat):
                load_batch(0)
                load_batch(1)
                p1n = [0] * steps
                p2n = [0] * steps
                i = 0
                while p2n[last] < NBLK and i < 600:
                    if i % 4 == 0:
                        load_batch(i // 4 + 2)
                    for s in range(steps):
                        drained = s > 0 and p2n[s - 1] >= NBLK
                        budget = 2 if drained else 1
                        for _ in range(budget):
                            if s == 0:
                                # paced by stage-0 input DMA batches
                                p1_ready = p1n[s] <= i
                            else:
                                # ring slots g..g+1 must be written AND
                                # halo-patched (issued) by stage s-1
                                need = min(4 * ((p1n[s] + 1) // 4) + 4, NBLK)
                                p1_ready = p2n[s - 1] >= need
                            if p1n[s] < NGRP and p1_ready:
                                part1(s, p1n[s])
                                p1n[s] += 1
                            if (
                                p2n[s] < NBLK
                                and (p2n[s] <= p1n[s] - 3 or p1n[s] >= NGRP)
                                and (s == last or p2n[s] < p2n[s + 1] + 15)
                            ):
                                part2(s, p2n[s])
                                p2n[s] += 1
                    i += 1
                assert p2n[last] >= NBLK, (p1n, p2n)

    nc.compile()
    return nc


_NC_CACHE = {}
_REPEAT = 1


def _get_nc(steps):
    key = (steps, _REPEAT)
    if key not in _NC_CACHE:
        _NC_CACHE[key] = _build_nc(steps, repeat=_REPEAT)
    return _NC_CACHE[key]


def _q8(a):
    return np.clip(a, -240, 240).astype(ml_dtypes.float8_e4m3)


def _fc1_scales(fc1_w):
    """Per-hidden-unit scales minimizing fp8 quantization error of
    (h*sig) and (fc1_w/sig), assuming half-normal h with rms 0.4."""
    rng = np.random.default_rng(42)
    hsamp = np.abs(rng.standard_normal(4096)) * 0.4
    Eh2 = (hsamp**2).mean()
    sig = np.zeros(HD)
    cands = 2.0 ** np.linspace(-4, 4, 65)
    for m in range(HD):
        w = fc1_w[m]
        best, berr = 1.0, np.inf
        for s in cands:
            qw = _q8(w / s).astype(np.float64) * s
            qh = _q8(hsamp * s).astype(np.float64) / s
            err = ((qh - hsamp) ** 2).mean() * np.sum(w**2) + Eh2 * np.sum(
                (qw - w) ** 2
            )
            if err < berr:
                best, berr = s, err
        sig[m] = best
    return sig


def _prep_weights(conv_w, conv_b, fc0_w, fc0_b, fc1_w):
    conv_w = np.asarray(conv_w, np.float64)[:, :, 0, :]  # [5,5,24]
    W1 = np.asarray(fc0_w, np.float64)[:C]  # [24,32]
    W2 = np.asarray(fc0_w, np.float64)[C:]  # [24,32]
    fc1_w = np.asarray(fc1_w, np.float64)  # [32,23]

    # M[ki, kj] = diag(conv_w[ki,kj]) @ W2 (+ W1 at center)
    M = conv_w[:, :, :, None] * W2[None, None, :, :]  # [5,5,24,32]
    M[2, 2] += W1

    WAB = np.zeros((2, 5, 128, 128), np.float64)
    for dj in range(5):
        for r in range(4):
            for f in range(4):
                ka = r - f  # di+2 for slab A (block g)
                if 0 <= ka <= 4:
                    WAB[0, dj, r * 32 : r * 32 + C, f * 32 : f * 32 + HD] = M[ka, dj]
                kb = r + 4 - f  # di+2 for slab B (block g+1)
                if 0 <= kb <= 4:
                    WAB[1, dj, r * 32 : r * 32 + C, f * 32 : f * 32 + HD] = M[kb, dj]

    # WAB8[k, dj, slab, m] = q8(512 * WAB[slab, dj, k, m])
    WAB8 = _q8(SCALE * WAB.transpose(2, 1, 0, 3))

    sig = _fc1_scales(fc1_w)
    # shift-absorbing fc1: out block k partitions r*32+c (rows 4k-2+r),
    # slab 0 = h(k-1) (rows 4k-4+f, f=r+2, r in 0,1),
    # slab 1 = h(k)   (rows 4k+f,   f=r-2, r in 2,3)
    WCS = np.zeros((128, 2, 128), np.float64)  # [q=f*32+m, slab, p=r*32+c]
    wq = _q8(fc1_w / sig[:, None]).astype(np.float64)  # [32,23]
    for r in (0, 1):
        f = r + 2
        for m in range(HD):
            WCS[f * 32 + m, 0, r * 32 + 1 : r * 32 + C] = wq[m]
    for r in (2, 3):
        f = r - 2
        for m in range(HD):
            WCS[f * 32 + m, 1, r * 32 + 1 : r * 32 + C] = wq[m]
    WCZ = WCS.copy()
    WCZ[:, 1, :] = 0.0
    WCS8 = WCS.astype(ml_dtypes.float8_e4m3)
    WCZ8 = WCZ.astype(ml_dtypes.float8_e4m3)

    bias_eff = np.asarray(fc0_b, np.float64) + np.asarray(conv_b, np.float64) @ W2
    sig4 = np.tile(sig, 4)
    BIAS = (np.tile(bias_eff, 4) * sig4).astype(np.float32).reshape(128, 1)
    SCL = (sig4 / SCALE).astype(np.float32).reshape(128, 1)
    return WAB8, WCS8, WCZ8, BIAS, SCL


def _prep_image(x_chw):
    """x_chw [C,H,W] f32 -> (X8D fp8 ring blocks, XSH bf16 shifted blocks)."""
    xp = np.zeros((CP, H + 4, W + 4), np.float32)
    xp[:C] = np.pad(x_chw, ((0, 0), (2, 2), (2, 2)), mode="reflect")
    x8 = _q8(xp)  # [32, 516, 516]

    X8D = np.zeros((NBATCH + 1, 4, CP, 4, RP), ml_dtypes.float8_e4m3)
    # block k covers padded rows 4k..4k+3 (= image rows 4k-2..4k+1)
    blk = x8.reshape(CP, NBLK, 4, W + 4).transpose(1, 2, 0, 3)  # [129,4,32,516]
    X8D[:NBATCH, :, :, :, : W + 4] = (
        blk[: 4 * NBATCH].reshape(NBATCH, 4, 4, CP, W + 4).transpose(0, 2, 3, 1, 4)
    )
    X8D[NBATCH, :, :, 0, : W + 4] = blk[4 * NBATCH]

    # bf16 shifted residual blocks (rows only padded, 512 cols)
    xr = np.zeros((CP, H + 4, W), np.float32)
    xr[:C] = np.pad(x_chw, ((0, 0), (2, 2), (0, 0)), mode="reflect")
    rblk = xr.reshape(CP, NBLK, 4, W).transpose(1, 2, 0, 3)  # [129,4,32,512]
    XSH = np.zeros((NBATCH + 1, 4, CP, 4, W), ml_dtypes.bfloat16)
    XSH[:NBATCH] = (
        rblk[: 4 * NBATCH].reshape(NBATCH, 4, 4, CP, W).transpose(0, 2, 3, 1, 4)
    )
    XSH[NBATCH, :, :, 0, :] = rblk[4 * NBATCH]
    return X8D, XSH


def _run_pass(x_chw, weights, steps):
    """One device invocation: `steps` NCA steps on x [B, C, H, W] f32."""
    WAB8, WCS8, WCZ8, BIAS, SCL = weights
    B = x_chw.shape[0]
    nc = _get_nc(steps)
    in_maps = []
    for i in range(NCORES):
        X8D, XSH = _prep_image(x_chw[i % B])
        in_maps.append({"X8D": X8D, "XSH": XSH, "WAB8": WAB8, "WCS": WCS8,
                        "WCZ": WCZ8, "BIAS": BIAS, "SCL": SCL})
    res = run_bass_kernel_spmd(nc, in_maps, core_ids=list(range(NCORES)))
    globals()["LAST_RESULTS"] = res
    out = np.empty((B, C, H, W), np.float32)
    for i in range(B):
        yd = np.asarray(res.results[i]["YD"])  # [33, 4, 32, 4, 512] bf16
        # blocks k=4j+s: [j,r,c,s,w] -> [k,r,c,w] -> rows 516
        arr = yd.transpose(0, 3, 1, 2, 4).reshape((NBATCH + 1) * 4, 4, CP, W)[:NBLK]
        y516 = arr.transpose(2, 0, 1, 3).reshape(CP, (NBLK) * 4, W)
        out[i] = y516[:C, 2 : 2 + H, :].astype(np.float32)
    return out


def kernel(x, conv_w, conv_b, fc0_w, fc0_b, fc1_w, steps):
    steps = int(steps)
    x = np.asarray(x, np.float32)
    B = x.shape[0]
    assert x.shape == (B, H, W, C) and 1 <= B <= NCORES, x.shape
    if steps <= 0:
        return x.copy()

    weights = _prep_weights(conv_w, conv_b, fc0_w, fc0_b, fc1_w)
    x_chw = np.ascontiguousarray(x.transpose(0, 3, 1, 2))
    # device pipeline supports 2 fused steps; decompose larger step counts
    while steps > 0:
        n = 2 if steps >= 2 else 1
        x_chw = _run_pass(x_chw, weights, n)
        steps -= n
    return np.ascontiguousarray(x_chw.transpose(0, 2, 3, 1)).astype(np.float32)


if __name__ == "__main__":
    rng = np.random.default_rng(0)
    inputs = {
        "x": rng.standard_normal((8, H, W, C), dtype=np.float32),
        "conv_w": (rng.standard_normal((5, 5, 1, C)) * 0.1).astype(np.float32),
        "conv_b": (rng.standard_normal((C,)) * 0.1).astype(np.float32),
        "fc0_w": (rng.standard_normal((2 * C, HD)) * 0.1).astype(np.float32),
        "fc0_b": (rng.standard_normal((HD,)) * 0.1).astype(np.float32),
        "fc1_w": (rng.standard_normal((HD, C - 1)) * 0.1).astype(np.float32),
        "steps": 2,
    }
    out = kernel(**inputs)
    print(out.shape, out.dtype)


# revision 44
# speedup vs baseline: 1.6650x; 1.6650x over previous
"""Trainium2 Bass kernel for a 2-step BasicNCA2D cell update (fp8 DoubleRow).

Strategy
--------
Data-parallel over batch: 8 images, one per NeuronCore. Per core the two NCA
steps are fused on-chip (x never round-trips to DRAM between steps).

Per step the math is
    y  = depthwise_conv5x5(x, conv_w) + conv_b        (reflect padding)
    h  = relu([x, y] @ fc0_w + fc0_b)
    dx = h @ fc1_w
    x' = concat([x[..., :1], x[..., 1:] + dx])

conv+fc0 are fused into accumulating fp8 DoubleRow matmuls over shifted
4-row x blocks (block k = image rows 4k-2..4k+1, 129 blocks/stage):
    h_pre[group g = rows 4g..4g+3] = sum_{dj} DR(WAB[dj], ring[k=g,g+1])
at 0.5 cycles/out-col -> 5 matmuls x 256 cycles per 4-row group.

NEW vs the previous version: the whole residual path lives in the SHIFTED
block layout, and fc1 also runs as one fp8 DoubleRow matmul per block:
  - h = relu(h_pre) is written by ACT directly as fp8 with a per-hidden-unit
    scale sigma_m folded into the relu scale/bias (per-partition APs), into a
    17-slot h ring (+ permanent zero slot 0, + dup slot 17 for pair wraps).
  - fc1 for shifted block k contracts K=256 = the (h(k-1), h(k)) slot pair in
    one DoubleRow matmul whose stationary absorbs the +-2-row misalignment:
    slab A uses only h(k-1) rows 2..3 (out rows 4k-2,4k-1), slab B only
    h(k) rows 0..1, with zero-padding elsewhere. fp8 weights are
    q8(fc1_w[m,:]/sigma_m).  107ns vs 213ns for the old bf16 fc1.
  - dx lands in SHIFTED-block PSUM, so the residual add (DVE) and the next
    stage's fp8 ring production (one aligned Pool tensor_copy per block,
    instead of two partition-shifted halves) are both single ops. x0's bf16
    residual is uploaded pre-shifted (XSH); the output is written shifted
    and un-shifted host-side.
  - fc1 outputs for adjacent blocks (2k, 2k+1) share one 2-bank PSUM tile so
    the residual add handles two blocks per DVE instruction (amortizing the
    PSUM access latency).

Engine budget per pass (cost model): PE 164us (conv 136 + fc1 28) is the
bottleneck; ACT 157us (256 relus), DVE ~154us (129 paired adds), Pool
~120us (65 paired ring copies + halos + dups). I/O: X8D fp8 ring upload,
XSH bf16 shifted residual, YD bf16 shifted output.

Accuracy: fp8 h/fc1 with weight-optimized per-unit scales adds ~1.1e-2 in
quadrature to the 1.30e-2 fp8-conv-path error -> 1.72e-2 (budget 2e-2),
validated in fp64 simulation over the full batch (steps=1: 1.05e-2).
"""

import numpy as np
import ml_dtypes

import concourse.mybir as mybir
import concourse.tile as tile
from concourse import bacc
from concourse.bass_utils import run_bass_kernel_spmd

F32 = mybir.dt.float32
F8 = mybir.dt.float8e4
BF16 = mybir.dt.bfloat16
DR = mybir.MatmulPerfMode.DoubleRow

H = 512
W = 512
C = 24
CP = 32
HD = 32
NCORES = 8
NGRP = H // 4          # 128 aligned conv groups of 4 rows per stage
NBLK = NGRP + 1        # 129 shifted blocks per stage
NBATCH = NBLK // 4     # 32 full 4-block DMA batches (+1 tail block)
RP = 520               # stage-0 ring slot pitch (516 used)
R1P = 516              # stage>=1 ring slot pitch
XS = 24                # residual/out ring slots (also ring0 wrap, +1 dup)
SCALE = 512.0


def _build_nc(steps: int, repeat: int = 1):
    nc = bacc.Bacc("TRN2", target_bir_lowering=False, debug=False)

    X8D = nc.dram_tensor("X8D", [NBATCH + 1, 4, CP, 4, RP], F8, kind="ExternalInput")
    XSH = nc.dram_tensor("XSH", [NBATCH + 1, 4, CP, 4, W], BF16, kind="ExternalInput")
    WAB8 = nc.dram_tensor("WAB8", [128, 5, 2, 128], F8, kind="ExternalInput")
    WCS = nc.dram_tensor("WCS", [128, 2, 128], F8, kind="ExternalInput")
    WCZ = nc.dram_tensor("WCZ", [128, 2, 128], F8, kind="ExternalInput")
    BIAS = nc.dram_tensor("BIAS", [128, 1], F32, kind="ExternalInput")
    SCL = nc.dram_tensor("SCL", [128, 1], F32, kind="ExternalInput")
    YD = nc.dram_tensor("YD", [NBATCH + 1, 4, CP, 4, W], BF16, kind="ExternalOutput")

    last = steps - 1

    with tile.TileContext(nc) as tc:
        with (
            tc.tile_pool(name="wpool", bufs=1) as wpool,
            tc.tile_pool(name="rpool", bufs=1) as rpool,
            tc.tile_pool(name="xpool", bufs=1) as xpool,
            tc.tile_pool(name="hpool", bufs=1) as hpool,
            tc.tile_pool(name="pp", bufs=4, space="PSUM") as pp,
            tc.tile_pool(name="ppdx", bufs=4, space="PSUM") as ppdx,
        ):
            wab_t = wpool.tile([128, 5, 2, 128], F8, tag="wab")
            nc.sync.dma_start(wab_t[:], WAB8.ap())
            wcs_t = wpool.tile([128, 2, 128], F8, tag="wcs")
            nc.sync.dma_start(wcs_t[:], WCS.ap())
            wcz_t = wpool.tile([128, 2, 128], F8, tag="wcz")
            nc.sync.dma_start(wcz_t[:], WCZ.ap())
            bias_t = wpool.tile([128, 1], F32, tag="bias")
            nc.sync.dma_start(bias_t[:], BIAS.ap())
            scl_t = wpool.tile([128, 1], F32, tag="scl")
            nc.sync.dma_start(scl_t[:], SCL.ap())

            # stage-0 x ring: 17-slot fp8 (slot16 = DMA'd dup of slot 0);
            # stage>=1 rings: linear 129 slots, fp8 of the shifted residual
            rings = [rpool.tile([128, XS + 1, RP], F8, tag="r0", name="r0")]
            for s in range(1, steps):
                rings.append(
                    rpool.tile([128, NBLK, R1P], F8, tag=f"r{s}", name=f"r{s}")
                )
            # h rings: slot 0 = permanent zeros, 1..16 rotate (h(g) at
            # 1 + g%16), slot 17 = dup of slot 1 for pair wraps
            hs = [
                hpool.tile([128, 18, W], F8, tag=f"h{s}", name=f"h{s}")
                for s in range(steps)
            ]
            # per-stage bf16 residual rings in SHIFTED block layout
            xsh = [
                xpool.tile([128, XS, W], BF16, tag=f"xsh{s}", name=f"xsh{s}")
                for s in range(steps)
            ]
            outb = xpool.tile([128, XS, W], BF16, tag="outb")

            for t in hs:
                nc.gpsimd.memset(t[:, 0, :], 0.0)
                nc.gpsimd.memset(t[:, 17, :], 0.0)

            def load_x8(j):
                """Stage-0 fp8 ring DMA: X8D blocks 4j..4j+3 (+dups)."""
                if j > NBATCH:
                    return
                if j < NBATCH:
                    s0 = (4 * j) % XS
                    nc.sync.dma_start(rings[0][:, s0 : s0 + 4, :], X8D.ap()[j])
                else:
                    st = (4 * NBATCH) % XS
                    nc.sync.dma_start(
                        rings[0][:, st, :], X8D.ap()[NBATCH, :, :, 0, :]
                    )
                if j > 0 and j % (XS // 4) == 0:
                    # block 4j lands in ring slot 0; duplicate into slot XS
                    nc.sync.dma_start(rings[0][:, XS, :], X8D.ap()[j, :, :, 0, :])

            def load_xsh(j):
                """Stage-0 bf16 residual DMA: XSH blocks 4j..4j+3."""
                if j > NBATCH:
                    return
                if j < NBATCH:
                    s0 = (4 * j) % XS
                    nc.sync.dma_start(xsh[0][:, s0 : s0 + 4, :], XSH.ap()[j])
                else:
                    st = (4 * NBATCH) % XS
                    nc.sync.dma_start(xsh[0][:, st, :], XSH.ap()[NBATCH, :, :, 0, :])

            def part1(s, g):
                """conv+fc0 (5 DoubleRow fp8 matmuls) + fp8 relu, group g."""
                BUILD_LOG.append((nc.next_id(), "p1", s, g))
                ring = rings[s]
                sg = g % XS if s == 0 else g
                hp = pp.tile([128, W], F32, tag="hp", name=f"hp{s}_{g}")
                for dj in range(5):
                    nc.tensor.matmul(
                        hp[:],
                        wab_t[:, dj],
                        ring[:, sg : sg + 2, dj : dj + W],
                        start=(dj == 0),
                        stop=(dj == 4),
                        perf_mode=DR,
                    )
                nc.scalar.activation(
                    hs[s][:, 1 + g % 16, :], hp[:],
                    mybir.ActivationFunctionType.Relu,
                    bias=bias_t[:], scale=scl_t[:],
                )

            def halo_cols_batch(r, s0, nslot):
                # reflect halo cols for ring slots s0..s0+nslot-1:
                # cols (0,514)<-(4,512) and (1,515)<-(3,511), strided APs
                src = r[:, s0 : s0 + nslot, :]
                for vc, pc in ((0, 4), (1, 3)):
                    nc.gpsimd.tensor_copy(
                        src[:, :, vc : vc + 515 : 514],
                        src[:, :, pc : pc + 509 : 508],
                    )

            def part2(s, k):
                """fc1 (shift-absorbing fp8 DR) + residual add (+ ring/DMA)."""
                BUILD_LOG.append((nc.next_id(), "p2", s, k))
                hr = hs[s]
                dxp = ppdx.tile([128, W], F32, tag="dx", name=f"dx{s}_{k}")
                if k >= 16 and k % 16 == 0 and k < NBLK - 1:
                    # wrap: h(k-1)@16 and h(k)@1 aren't adjacent; use two
                    # accumulating matmuls with half-zeroed weights (slab A
                    # of the first reads the zero slot, slab B of the
                    # second reads stale slot 17 times zero weights)
                    nc.tensor.matmul(
                        dxp[:], wcs_t, hr[:, 0:2, :],
                        start=True, stop=False, perf_mode=DR,
                    )
                    nc.tensor.matmul(
                        dxp[:], wcz_t, hr[:, 16:18, :],
                        start=False, stop=True, perf_mode=DR,
                    )
                else:
                    ps = 0 if k == 0 else 1 + ((k - 1) % 16)
                    wt = wcz_t if k == NBLK - 1 else wcs_t
                    nc.tensor.matmul(
                        dxp[:], wt, hr[:, ps : ps + 2, :],
                        start=True, stop=True, perf_mode=DR,
                    )
                sk = k % XS
                dst = outb if s == last else xsh[s + 1]
                nc.vector.tensor_add(
                    dst[:, sk, :], dxp[:], xsh[s][:, sk, :]
                )
                if s < last:
                    r = rings[s + 1]
                    # fp8 ring production: paired copy once both blocks exist
                    if k % 2 == 1:
                        k0 = k - 1
                        s0 = k0 % XS
                        nc.gpsimd.tensor_copy(
                            r[:, k0 : k0 + 2, 2 : 2 + W], dst[:, s0 : s0 + 2, :]
                        )
                    elif k == NBLK - 1:
                        nc.gpsimd.tensor_copy(
                            r[:, k, 2 : 2 + W], dst[:, sk, :]
                        )
                    if k % 4 == 3:
                        halo_cols_batch(r, k - 3, 4)
                        if k == 3:
                            # block 0 reflect rows: row -2 := row 2 (block 1
                            # parts 0:32), row -1 := row 1 (own parts 96:128)
                            nc.gpsimd.tensor_copy(
                                r[0:32, 0, 0:R1P], r[0:32, 1, 0:R1P]
                            )
                            nc.gpsimd.tensor_copy(
                                r[32:64, 0, 0:R1P], r[96:128, 0, 0:R1P]
                            )
                    if k == NBLK - 1:
                        kb = NBLK - 1
                        halo_cols_batch(r, kb, 1)
                        # block 128 reflect rows: 512 := 510, 513 := 509
                        nc.gpsimd.tensor_copy(
                            r[64:96, kb, 0:R1P], r[0:32, kb, 0:R1P]
                        )
                        nc.gpsimd.tensor_copy(
                            r[96:128, kb, 0:R1P], r[96:128, kb - 1, 0:R1P]
                        )
                else:
                    if k >= NBLK - 5:
                        # tail blocks: per-block DMAs to shorten the drain
                        nc.sync.dma_start(
                            YD.ap()[k // 4, :, :, k % 4, :],
                            outb[:, k % XS, :],
                        )
                    elif k % 4 == 3:
                        sf = (k - 3) % XS
                        nc.sync.dma_start(
                            YD.ap()[k // 4], outb[:, sf : sf + 4, :]
                        )

            for _rep in range(repeat):
                # PE p-state warmup: throwaway matmuls on the weight tile
                # while the first input batches stream in (PE reaches full
                # clock after ~3us of continuous execution)
                for wj in range(6):
                    wtile = ppdx.tile([128, W], F32, tag="dx", name=f"wm{wj}")
                    nc.tensor.matmul(
                        wtile[:], wab_t[:, 0, 0], wab_t[:, 0:4, 0, :],
                        start=True, stop=True,
                    )
                for jj in range(3):
                    load_x8(jj)
                    load_xsh(jj)
                p1n = [0] * steps
                p2n = [0] * steps
                i = 0
                while p2n[last] < NBLK and i < 600:
                    if i % 2 == 0:
                        load_x8(i // 2 + 3)
                        load_xsh(i // 2 + 3)
                    for s in range(steps):
                        for _ in range(2):
                            if s == 0:
                                # paced by stage-0 input DMA batches
                                p1_ready = p1n[s] <= 2 * i
                            else:
                                # ring slots g..g+1 must be written AND
                                # halo-patched by stage s-1, with enough lag
                                # that Pool has drained the copy queue
                                need = min(
                                    NBLK,
                                    max(4 * ((p1n[s] + 1) // 4) + 4,
                                        p1n[s] + 15),
                                )
                                p1_ready = p2n[s - 1] >= need
                            if p1n[s] < NGRP and p1_ready:
                                part1(s, p1n[s])
                                p1n[s] += 1
                            if (
                                p2n[s] < NBLK
                                and (p2n[s] <= p1n[s] - (3 if s == 0 else 4) or p1n[s] >= NGRP)
                                and (s == last or p2n[s] < p2n[s + 1] + XS - 1)
                            ):
                                part2(s, p2n[s])
                                p2n[s] += 1
                    i += 1
                assert p2n[last] >= NBLK, (p1n, p2n)

    nc.compile()
    return nc


BUILD_LOG = []

_NC_CACHE = {}
_REPEAT = 1


def _get_nc(steps):
    key = (steps, _REPEAT)
    if key not in _NC_CACHE:
        _NC_CACHE[key] = _build_nc(steps, repeat=_REPEAT)
    return _NC_CACHE[key]


def _q8(a):
    return np.clip(a, -240, 240).astype(ml_dtypes.float8_e4m3)


def _fc1_scales(fc1_w):
    """Per-hidden-unit scales minimizing fp8 quantization error of
    (h*sig) and (fc1_w/sig), assuming half-normal h with rms 0.4."""
    rng = np.random.default_rng(42)
    hsamp = np.abs(rng.standard_normal(4096)) * 0.4
    Eh2 = (hsamp**2).mean()
    sig = np.zeros(HD)
    cands = 2.0 ** np.linspace(-4, 4, 65)
    for m in range(HD):
        w = fc1_w[m]
        best, berr = 1.0, np.inf
        for s in cands:
            qw = _q8(w / s).astype(np.float64) * s
            qh = _q8(hsamp * s).astype(np.float64) / s
            err = ((qh - hsamp) ** 2).mean() * np.sum(w**2) + Eh2 * np.sum(
                (qw - w) ** 2
            )
            if err < berr:
                best, berr = s, err
        sig[m] = best
    return sig


def _prep_weights(conv_w, conv_b, fc0_w, fc0_b, fc1_w):
    conv_w = np.asarray(conv_w, np.float64)[:, :, 0, :]  # [5,5,24]
    W1 = np.asarray(fc0_w, np.float64)[:C]  # [24,32]
    W2 = np.asarray(fc0_w, np.float64)[C:]  # [24,32]
    fc1_w = np.asarray(fc1_w, np.float64)  # [32,23]

    # M[ki, kj] = diag(conv_w[ki,kj]) @ W2 (+ W1 at center)
    M = conv_w[:, :, :, None] * W2[None, None, :, :]  # [5,5,24,32]
    M[2, 2] += W1

    WAB = np.zeros((2, 5, 128, 128), np.float64)
    for dj in range(5):
        for r in range(4):
            for f in range(4):
                ka = r - f  # di+2 for slab A (block g)
                if 0 <= ka <= 4:
                    WAB[0, dj, r * 32 : r * 32 + C, f * 32 : f * 32 + HD] = M[ka, dj]
                kb = r + 4 - f  # di+2 for slab B (block g+1)
                if 0 <= kb <= 4:
                    WAB[1, dj, r * 32 : r * 32 + C, f * 32 : f * 32 + HD] = M[kb, dj]

    # WAB8[k, dj, slab, m] = q8(512 * WAB[slab, dj, k, m])
    WAB8 = _q8(SCALE * WAB.transpose(2, 1, 0, 3))

    sig = _fc1_scales(fc1_w)
    # shift-absorbing fc1: out block k partitions r*32+c (rows 4k-2+r),
    # slab 0 = h(k-1) (rows 4k-4+f, f=r+2, r in 0,1),
    # slab 1 = h(k)   (rows 4k+f,   f=r-2, r in 2,3)
    WCS = np.zeros((128, 2, 128), np.float64)  # [q=f*32+m, slab, p=r*32+c]
    wq = _q8(fc1_w / sig[:, None]).astype(np.float64)  # [32,23]
    for r in (0, 1):
        f = r + 2
        for m in range(HD):
            WCS[f * 32 + m, 0, r * 32 + 1 : r * 32 + C] = wq[m]
    for r in (2, 3):
        f = r - 2
        for m in range(HD):
            WCS[f * 32 + m, 1, r * 32 + 1 : r * 32 + C] = wq[m]
    WCZ = WCS.copy()
    WCZ[:, 1, :] = 0.0
    WCS8 = WCS.astype(ml_dtypes.float8_e4m3)
    WCZ8 = WCZ.astype(ml_dtypes.float8_e4m3)

    bias_eff = np.asarray(fc0_b, np.float64) + np.asarray(conv_b, np.float64) @ W2
    sig4 = np.tile(sig, 4)
    BIAS = (np.tile(bias_eff, 4) * sig4).astype(np.float32).reshape(128, 1)
    SCL = (sig4 / SCALE).astype(np.float32).reshape(128, 1)
    return WAB8, WCS8, WCZ8, BIAS, SCL


def _prep_image(x_chw):
    """x_chw [C,H,W] f32 -> (X8D fp8 ring blocks, XSH bf16 shifted blocks)."""
    xp = np.zeros((CP, H + 4, W + 4), np.float32)
    xp[:C] = np.pad(x_chw, ((0, 0), (2, 2), (2, 2)), mode="reflect")
    x8 = _q8(xp)  # [32, 516, 516]

    X8D = np.zeros((NBATCH + 1, 4, CP, 4, RP), ml_dtypes.float8_e4m3)
    # block k covers padded rows 4k..4k+3 (= image rows 4k-2..4k+1)
    blk = x8.reshape(CP, NBLK, 4, W + 4).transpose(1, 2, 0, 3)  # [129,4,32,516]
    X8D[:NBATCH, :, :, :, : W + 4] = (
        blk[: 4 * NBATCH].reshape(NBATCH, 4, 4, CP, W + 4).transpose(0, 2, 3, 1, 4)
    )
    X8D[NBATCH, :, :, 0, : W + 4] = blk[4 * NBATCH]

    # bf16 shifted residual blocks (rows only padded, 512 cols)
    xr = np.zeros((CP, H + 4, W), np.float32)
    xr[:C] = np.pad(x_chw, ((0, 0), (2, 2), (0, 0)), mode="reflect")
    rblk = xr.reshape(CP, NBLK, 4, W).transpose(1, 2, 0, 3)  # [129,4,32,512]
    XSH = np.zeros((NBATCH + 1, 4, CP, 4, W), ml_dtypes.bfloat16)
    XSH[:NBATCH] = (
        rblk[: 4 * NBATCH].reshape(NBATCH, 4, 4, CP, W).transpose(0, 2, 3, 1, 4)
    )
    XSH[NBATCH, :, :, 0, :] = rblk[4 * NBATCH]
    return X8D, XSH


def _run_pass(x_chw, weights, steps):
    """One device invocation: `steps` NCA steps on x [B, C, H, W] f32."""
    WAB8, WCS8, WCZ8, BIAS, SCL = weights
    B = x_chw.shape[0]
    nc = _get_nc(steps)
    in_maps = []
    for i in range(NCORES):
        X8D, XSH = _prep_image(x_chw[i % B])
        in_maps.append({"X8D": X8D, "XSH": XSH, "WAB8": WAB8, "WCS": WCS8,
                        "WCZ": WCZ8, "BIAS": BIAS, "SCL": SCL})
    res = run_bass_kernel_spmd(nc, in_maps, core_ids=list(range(NCORES)))
    globals()["LAST_RESULTS"] = res
    out = np.empty((B, C, H, W), np.float32)
    for i in range(B):
        yd = np.asarray(res.results[i]["YD"])  # [33, 4, 32, 4, 512] bf16
        # blocks k=4j+s: [j,r,c,s,w] -> [k,r,c,w] -> rows 516
        arr = yd.transpose(0, 3, 1, 2, 4).reshape((NBATCH + 1) * 4, 4, CP, W)[:NBLK]
        y516 = arr.transpose(2, 0, 1, 3).reshape(CP, (NBLK) * 4, W)
        out[i] = y516[:C, 2 : 2 + H, :].astype(np.float32)
    return out


def kernel(x, conv_w, conv_b, fc0_w, fc0_b, fc1_w, steps):
    steps = int(steps)
    x = np.asarray(x, np.float32)
    B = x.shape[0]
    assert x.shape == (B, H, W, C) and 1 <= B <= NCORES, x.shape
    if steps <= 0:
        return x.copy()

    weights = _prep_weights(conv_w, conv_b, fc0_w, fc0_b, fc1_w)
    x_chw = np.ascontiguousarray(x.transpose(0, 3, 1, 2))
    # device pipeline supports 2 fused steps; decompose larger step counts
    while steps > 0:
        n = 2 if steps >= 2 else 1
        x_chw = _run_pass(x_chw, weights, n)
        steps -= n
    return np.ascontiguousarray(x_chw.transpose(0, 2, 3, 1)).astype(np.float32)


if __name__ == "__main__":
    rng = np.random.default_rng(0)
    inputs = {
        "x": rng.standard_normal((8, H, W, C), dtype=np.float32),
        "conv_w": (rng.standard_normal((5, 5, 1, C)) * 0.1).astype(np.float32),
        "conv_b": (rng.standard_normal((C,)) * 0.1).astype(np.float32),
        "fc0_w": (rng.standard_normal((2 * C, HD)) * 0.1).astype(np.float32),
        "fc0_b": (rng.standard_normal((HD,)) * 0.1).astype(np.float32),
        "fc1_w": (rng.standard_normal((HD, C - 1)) * 0.1).astype(np.float32),
        "steps": 2,
    }
    out = kernel(**inputs)
    print(out.shape, out.dtype)
